# revision 32
# baseline (speedup 1.0000x reference)
"""v5: v2's slot-ring with ±y instead of y (no 8-row evacuations).

X slots (32 rows, ring of 4): rows 0-15 = relu(+y),relu(-y), rows 16-23 =
noise, 24-31 pad.  z(s,sg) = ONE full-K matmul (sign-folded lhsT [128,80]).
y-MM emits [+y;-y] (16 rows, b2 via ones row) into psum bank sg rows 0-15
(bank freed by the z relu).  pm-relu moves 4 banks at once: [16, 2048]
psum -> X slot rows (one ACT op for streams 0-3, one DVE op for 4-7).
relu(+y)-relu(-y) == y exactly; the host combines the planes.  DMAs are
one plain [8, 4096] noise-in and one [16, 4096] gen-out per superwave.
"""

import numpy as np

NN, KP, NH, W = 64, 4, 10, 512
NS, CS = 8, 8
B_SHARD = NS * CS * W
N_CORES = 8
NSW = CS + NN - 1
NSLOT = 4
ZROWS = 8 * NH
HONES = 96
NLAG = 2
OLAG = 1


def window(s):
    return range(max(0, s - CS + 1), min(NN - 1, s) + 1)


def w1_row_for_parent(n, j):
    return KP - j if n >= KP else n - j


def slotbase(s):
    return 32 * (s % NSLOT)


def hcol(s, sg):
    return ((s % 2) * 8 + sg) * W


def pack_v5(W1, b1, W2, b2):
    W1 = np.asarray(W1, np.float32)
    b1 = np.asarray(b1, np.float32)
    W2 = np.asarray(W2, np.float32)
    b2 = np.asarray(b2, np.float32)
    WZ = np.zeros((128, NSW * ZROWS), np.float32)
    WY = np.zeros((128, NSW * 16), np.float32)
    B1T = np.zeros((128, NSW), np.float32)
    for s in range(NSW):
        for n in window(s):
            c0 = s * ZROWS + NH * (n % 8)
            for j in range(1, KP + 1):
                m = n - j
                if m < 0:
                    continue
                wv = W1[n, w1_row_for_parent(n, j)]
                r = slotbase(s - j)
                WZ[r + (m % 8), c0:c0 + NH] = wv
                WZ[r + 8 + (m % 8), c0:c0 + NH] = -wv
            WZ[slotbase(s) + 16 + (n % 8), c0:c0 + NH] = W1[n, KP]
            B1T[NH * (n % 8):NH * (n % 8) + NH, s] = b1[n]
            cy = s * 16 + (n % 8)
            WY[NH * (n % 8):NH * (n % 8) + NH, cy] = W2[n]
            WY[HONES, cy] = b2[n]
            WY[NH * (n % 8):NH * (n % 8) + NH, cy + 8] = -W2[n]
            WY[HONES, cy + 8] = -b2[n]
    return WZ, WY, B1T


def emulate_core(noiseT, WZ, WY, B1T):
    """Numpy mirror; returns genpm [NSW, 16, NS*W]."""
    X = np.zeros((128, NS * W), np.float32)
    Hb = np.zeros((128, 2 * 8 * W), np.float32)
    Hb[HONES, :] = 1.0
    genpm = np.zeros((NSW, 16, NS * W), np.float32)

    def noise_in(s):
        if s >= NSW:
            return
        r0 = slotbase(s) + 16
        X[r0:r0 + 8, :] = 0.0
        for n in window(s):
            c = s - n
            for sg in range(NS):
                X[r0 + (n % 8), sg * W:(sg + 1) * W] = \
                    noiseT[n, (sg * CS + c) * W:(sg * CS + c + 1) * W]

    for sp in range(NLAG):
        noise_in(sp)
    for s in range(NSW):
        noise_in(s + NLAG)
        for sg in range(NS):
            z = WZ[:, s * ZROWS:(s + 1) * ZROWS].T @ X[:, sg * W:(sg + 1) * W]
            hc = hcol(s, sg)
            Hb[:ZROWS, hc:hc + W] = \
                np.maximum(z + B1T[:ZROWS, s:s + 1], 0.0)
            ypm = WY[:HONES + 1, s * 16:(s + 1) * 16].T @ \
                Hb[:HONES + 1, hc:hc + W]                     # [16, W]
            X[slotbase(s):slotbase(s) + 16, sg * W:(sg + 1) * W] = \
                np.maximum(ypm, 0.0)
        genpm[s] = X[slotbase(s):slotbase(s) + 16, :]
    return genpm


def build_bass():
    import concourse.bass as bass
    import concourse.bacc as bacc
    import concourse.mybir as mybir
    import concourse.tile as tile

    f32 = mybir.dt.float32
    bf16 = mybir.dt.bfloat16
    RELU = mybir.ActivationFunctionType.Relu
    ADD = mybir.AluOpType.add
    MAX = mybir.AluOpType.max

    nc = bacc.Bacc("TRN2", target_bir_lowering=False, debug=False,
                   enable_asserts=False, num_devices=N_CORES)

    d_noise = nc.dram_tensor("noiseS", [NSW, 8, NS * W], bf16,
                             kind="ExternalInput").ap()
    d_wz = nc.dram_tensor("WZ", [128, NSW * ZROWS], bf16,
                          kind="ExternalInput").ap()
    d_wy = nc.dram_tensor("WY", [128, NSW * 16], bf16,
                          kind="ExternalInput").ap()
    d_b1 = nc.dram_tensor("B1T", [128, NSW], f32,
                          kind="ExternalInput").ap()
    d_gen = nc.dram_tensor("genpm", [NSW, 16, NS * W], bf16,
                           kind="ExternalOutput").ap()

    with tile.TileContext(nc) as tc:
        with tc.tile_pool(name="sb", bufs=1) as sb, \
             tc.tile_pool(name="ps", bufs=1, space="PSUM") as pp:
            X = sb.tile([128, NS * W], bf16)
            Hb = sb.tile([128, 2 * 8 * W], bf16)
            WZ = sb.tile([128, NSW * ZROWS], bf16)
            WY = sb.tile([128, NSW * 16], bf16)
            B1T = sb.tile([128, NSW], f32)
            zP = pp.tile([128, 8 * W], f32, name="zP")

            nc.sync.dma_start(WZ[:], d_wz[:])
            nc.sync.dma_start(WY[:], d_wy[:])
            nc.sync.dma_start(B1T[:], d_b1[:])
            nc.vector.memset(X[:], 0.0)
            nc.vector.memset(Hb[:], 0.0)
            nc.vector.memset(Hb[HONES:HONES + 1, :], 1.0)
            nc.vector.memset(zP[:], 0.0)

            def noise_in(s):
                if s >= NSW:
                    return
                src = bass.AP(d_noise.tensor, s * 8 * NS * W,
                              [[NS * W, 8], [1, NS * W]])
                r0 = slotbase(s) + 16
                nc.sync.dma_start(X[r0:r0 + 8, :], src)

            def gen_out(s):
                dst = bass.AP(d_gen.tensor, s * 16 * NS * W,
                              [[NS * W, 16], [1, NS * W]])
                nc.sync.dma_start(dst, X[slotbase(s):slotbase(s) + 16, :])

            for sp in range(NLAG):
                noise_in(sp)
            for s in range(NSW):
                noise_in(s + NLAG)
                for sg in range(NS):
                    nc.tensor.matmul(
                        zP[:ZROWS, sg * W:(sg + 1) * W],
                        WZ[:, s * ZROWS:(s + 1) * ZROWS],
                        X[:, sg * W:(sg + 1) * W],
                        start=True, stop=True, skip_group_check=True)

                def relu_pair(sg0, eng):
                    dst = Hb[:ZROWS, hcol(s, sg0):hcol(s, sg0) + 2 * W]
                    src = zP[:ZROWS, sg0 * W:(sg0 + 2) * W]
                    if eng == 0:
                        nc.scalar.activation(dst, src, RELU,
                                             bias=B1T[:ZROWS, s:s + 1])
                    else:
                        nc.vector.tensor_scalar(dst, src,
                                                B1T[:ZROWS, s:s + 1],
                                                0.0, ADD, MAX)

                relu_pair(0, 0)
                relu_pair(2, 1)
                relu_pair(4, 0)
                relu_pair(6, 1)

                sb0 = slotbase(s)
                for sg in range(NS):
                    nc.tensor.matmul(
                        zP[:16, sg * W:(sg + 1) * W],
                        WY[:HONES + 1, s * 16:(s + 1) * 16],
                        Hb[:HONES + 1, hcol(s, sg):hcol(s, sg) + W],
                        start=True, stop=True, skip_group_check=True)
                    # pm quarter-relus alternate ACT/DVE right after their
                    # two y matmuls, so the piece feeding the next
                    # superwave's earliest z matmuls lands first and no
                    # engine queues two pieces back-to-back.
                    if sg % 2 == 1:
                        q = sg // 2
                        dst = X[sb0:sb0 + 16, q * 2 * W:(q + 1) * 2 * W]
                        src = zP[:16, q * 2 * W:(q + 1) * 2 * W]
                        if q % 2 == 0:
                            nc.scalar.activation(dst, src, RELU)
                        else:
                            nc.vector.tensor_scalar(dst, src, 0.0, 0.0,
                                                    ADD, MAX)
                if s - OLAG >= 0:
                    gen_out(s - OLAG)
            for s in range(max(0, NSW - OLAG), NSW):
                gen_out(s)
    return nc


_COMPILED = None
TRACE = False
LAST = None


def kernel(**inputs):
    global _COMPILED, LAST
    noise = np.asarray(inputs["noise"], np.float32)
    WZ, WY, B1T = pack_v5(inputs["W1"], inputs["b1"], inputs["W2"],
                          inputs["b2"])
    if _COMPILED is None:
        nc = build_bass()
        nc.compile()
        _COMPILED = nc
    nc = _COMPILED

    import ml_dtypes
    bfnp = ml_dtypes.bfloat16
    noiseT = np.ascontiguousarray(noise.T)
    wz16, wy16 = WZ.astype(bfnp), WY.astype(bfnp)
    in_maps = []
    for core in range(N_CORES):
        nt = noiseT[:, core * B_SHARD:(core + 1) * B_SHARD]
        ntc = nt.reshape(NN, NS, CS, W)
        ns = np.zeros((NSW, 8, NS, W), np.float32)
        for s in range(NSW):
            for n in window(s):
                ns[s, n % 8, :, :] = ntc[n, :, s - n, :]
        in_maps.append(dict(noiseS=ns.reshape(NSW, 8, NS * W).astype(bfnp),
                            WZ=wz16, WY=wy16, B1T=B1T))

    from concourse.bass_utils import run_bass_kernel_spmd
    res = run_bass_kernel_spmd(nc, in_maps, core_ids=list(range(N_CORES)),
                               trace=TRACE)
    LAST = res
    gen = np.empty((noise.shape[0], NN), np.float32)
    for core in range(N_CORES):
        pm = np.asarray(res.results[core]["genpm"], np.float32)
        pmv = pm.reshape(NSW, 2, 8, NS, W)           # [s, pl, r, sg, w]
        yy = np.maximum(pmv[:, 0], 0.0) - np.maximum(pmv[:, 1], 0.0)
        g = np.empty((NN, NS, CS, W), np.float32)
        for n in range(NN):
            for c in range(CS):
                g[n, :, c, :] = yy[n + c, n % 8, :, :]
        gen[core * B_SHARD:(core + 1) * B_SHARD, :] = \
            g.reshape(NN, B_SHARD).T
    return gen
